# revision 19
# baseline (speedup 1.0000x reference)
"""Bahdanau additive attention on 8 trn2 NeuronCores.

Math (per batch b):
  Qp = Q[b] @ Wq.T            [128, 256]
  Kp = K[b] @ Wk.T            [1024, 256]
  v  = attention / ||attention|| * scalar_constant
  s[q,k]   = sum_h v_h * tanh(Qp[q,h] + Kp[k,h] + bias_h)
  probs    = softmax_k(s)
  context  = probs @ Kp

Sharding: 8 cores = (4 batches) x (2 query-halves of 64).

Per-core device strategy (layout: h on partitions, k on free dim):
  - PE: Kp^T/Qp^T projections (Kp in float32r), score reduction over h
    as M=1 bf16 matmuls packed 4-per-group via tile_position col-groups,
    probs^T/Kp transposes, context matmul.
  - The 64*1024*256 elementwise tensor is split at k=M_SPLIT:
      k < M_SPLIT : ACT tanh(Kp^T[:,k] + Qp_col_q) (bias fused), bf16 out
      k >= M_SPLIT: DVE fused custom op: ~1/(Ea*Eb + 1) in ONE pass
        (bitwise-NOT seed + 1 Newton step; Ea=e^{2Qp} as the per-partition
        scalar, Eb=e^{2Kp} precomputed), bf16 out; tanh = 1 - 2R with the
        additive constant sum_h a_h compensated in the softmax exp bias.
  - softmax needs no max-subtraction (|scores| <= 1 provably); the
    normalization scal/||a|| is applied via the ACT-exp `scale` operand,
    so the attention vector itself is used unnormalized as matmul lhsT.
  - PSUM rule: a start=True matmul clears accumulation state for its
    whole (partition-row x bank) => the two routes' accumulation groups
    live in separate PSUM tiles (disjoint banks).
"""

import os
import sys

sys.path.insert(0, "/opt/trn_rl_repo")

import numpy as np

import concourse.bass as bass
import concourse.bacc as bacc
import concourse.tile as tile
from concourse import mybir
from concourse.masks import make_identity
from concourse.bass_utils import run_bass_kernel_spmd

FP32 = mybir.dt.float32
BF16 = mybir.dt.bfloat16
F32R = mybir.dt.float32r
AX = mybir.AxisListType
ALU = mybir.AluOpType
ACTF = mybir.ActivationFunctionType

B, QS, KS = 4, 128, 1024
D, H = 1024, 256
QC = 64           # queries per core
NCORES = 8
M_SPLIT = int(os.environ.get("ATTN_M_SPLIT", "512"))  # k cols on ACT route

# 1-Newton-step approx of 1/(x*s+1), minimax-tuned (max rel err ~1.8e-3,
# below the bf16 output rounding)
MR_SEED = -0.2358
MR_NR = 2.0018334

LAST_RESULTS = None  # test harness peeks at this for exec_time_ns


def _register_fused_recip():
    """Register the custom DVE op out = approx(1/(Src0*s0 + 1)) at runtime."""
    import concourse.dve_ops as dvo
    from concourse.dve_spec import Spec, Src0, C0, C1, C2, One, AluOp, Bin, lower
    from concourse.dve_uop import DveOpSpec

    name = "MADD_RECIP1_ANT"
    for o in dvo.OPS:
        if o.name == name:
            return o
    P = Src0 * C0 + One
    nP = Bin(AluOp.BITWISE_NOT, P, P)
    z0 = nP * C1

    def _ref(in0, in1, c0, c1, c2):
        Pv = (in0 * c0 + 1.0).astype(np.float32)
        nPv = (~Pv.view(np.int32)).view(np.float32)
        z = nPv * np.float32(c1)
        return z * (np.float32(c2) - Pv * z)

    spec = Spec(body=z0 * (C2 - P * z0), reference=_ref)
    row = dvo._CUSTOM_DVE_ROW_BASE + len(dvo.OPS)
    shas = {}
    for ver in ("v3", "v4"):
        s = DveOpSpec(name=name, opcode=row, uops=lower(spec, ver=ver),
                      rd1_en=False)
        shas[ver] = s.sha(ver)
    op = dvo.DveOp(name, spec, subdim=False, uops_sha=shas)
    dvo.OPS.append(op)
    dvo.CUSTOM_DVE_SPECS[name] = spec
    dvo._SUB_OPCODE_FOR_NAME[name] = row
    return op


def _segments(lo, hi):
    """Split [lo,hi) at 512-aligned PSUM bank boundaries."""
    segs = []
    while lo < hi:
        nxt = min(hi, (lo // 512 + 1) * 512)
        segs.append((lo, nxt))
        lo = nxt
    return segs


def build_nc(m=M_SPLIT):
    fused_op = _register_fused_recip()
    nc = bacc.Bacc("TRN2", target_bir_lowering=False, debug=False)
    w = KS - m

    kt_e = nc.dram_tensor("KT", [D, KS], F32R, kind="ExternalInput")
    wkt_e = nc.dram_tensor("WkT", [D, H], F32R, kind="ExternalInput")
    wqt_e = nc.dram_tensor("WqT", [D, H], FP32, kind="ExternalInput")
    qt_e = nc.dram_tensor("QT", [D, QC], FP32, kind="ExternalInput")
    att_e = nc.dram_tensor("attc", [128, 2], FP32, kind="ExternalInput")
    bias_e = nc.dram_tensor("biasc", [128, 2], FP32, kind="ExternalInput")
    scal_e = nc.dram_tensor("scal", [1, 1], FP32, kind="ExternalInput")
    probs_e = nc.dram_tensor("probs", [QC, KS], FP32, kind="ExternalOutput")
    ctx_e = nc.dram_tensor("ctx", [QC, H], FP32, kind="ExternalOutput")
    dbg_scores_e = None
    if os.environ.get("ATTN_DEBUG_SCORES"):
        dbg_scores_e = nc.dram_tensor("dbg_scores", [QC, KS], FP32,
                                      kind="ExternalOutput")

    with tile.TileContext(nc) as tc:
        with (
            tc.tile_pool(name="singles", bufs=1) as singles,
            tc.tile_pool(name="ta", bufs=8) as pool_a,
            tc.tile_pool(name="tb", bufs=8) as pool_b,
            tc.tile_pool(name="ps_sc", bufs=2, space="PSUM") as ps_sc,
            tc.tile_pool(name="ps_sm", bufs=3, space="PSUM") as ps_sm,
        ):
            # ---------------- constants / params ----------------
            ident = singles.tile([128, 128], FP32)
            make_identity(nc, ident[:])
            ones_col = singles.tile([128, 1], FP32)
            nc.vector.memset(ones_col[:], 1.0)
            ones_row = singles.tile([1, 128], FP32)
            nc.vector.memset(ones_row[:], 1.0)

            att_sb = singles.tile([128, 2], FP32)
            nc.sync.dma_start(out=att_sb[:], in_=att_e[:])
            bias_sb = singles.tile([128, 2], FP32)
            nc.sync.dma_start(out=bias_sb[:], in_=bias_e[:])
            scal_sb = singles.tile([1, 1], FP32)
            nc.sync.dma_start(out=scal_sb[:], in_=scal_e[:])

            # small inputs first so the Qp/Ea chain completes early
            qt_sb = singles.tile([128, 8, QC], FP32)
            nc.sync.dma_start(out=qt_sb[:],
                              in_=qt_e.rearrange("(n p) q -> p n q", p=128))
            wqt_sb = singles.tile([128, 8, H], FP32)
            nc.sync.dma_start(out=wqt_sb[:],
                              in_=wqt_e.rearrange("(n p) h -> p n h", p=128))
            wkt_sb = singles.tile([128, 8, H], F32R)
            nc.sync.dma_start(out=wkt_sb[:],
                              in_=wkt_e.rearrange("(n p) h -> p n h", p=128))
            kt_h0 = singles.tile([128, 8, 512], F32R)
            kt_h1 = singles.tile([128, 8, 512], F32R)
            kt_v = kt_e.rearrange("(n p) k -> p n k", p=128)
            for dch in range(8):
                nc.sync.dma_start(out=kt_h1[:, dch, :],
                                  in_=kt_v[:, dch, 512:1024])
            for dch in range(8):
                nc.sync.dma_start(out=kt_h0[:, dch, :],
                                  in_=kt_v[:, dch, 0:512])

            # DVE-domain copy of attention (keeps DVE ops single-wait)
            att_v = singles.tile([128, 2], FP32)
            nc.vector.tensor_copy(att_v[:], att_sb[:])
            # attneg2 = -2 * attention, bf16 (route-B lhsT)
            attneg2 = singles.tile([128, 2], BF16)
            nc.vector.tensor_scalar_mul(attneg2[:], att_v[:], -2.0)
            # bf16 copy of attention (route-A lhsT)
            att_r = singles.tile([128, 2], BF16)
            nc.scalar.copy(att_r[:], att_sb[:])

            # ||a||^2 = sum(a^2) ; vs = scal/sqrt(.) with one Newton step.
            # Scalar chain runs on ACT (scale/bias APs) + DVE reciprocal only,
            # so no DVE op joins two foreign semaphores.
            scal_a = singles.tile([1, 1], FP32)
            nc.scalar.copy(scal_a[:], scal_sb[:])
            asq = singles.tile([128, 2], FP32)
            nc.scalar.square(asq[:], att_sb[:])
            asq_r = singles.tile([128, 1], FP32)
            nc.vector.tensor_reduce(asq_r[:], asq[:], axis=AX.X, op=ALU.add)
            n2_ps = ps_sm.tile([1, 1], FP32, tag="ps")
            nc.tensor.matmul(n2_ps[:], lhsT=asq_r[:], rhs=ones_col[:],
                             start=True, stop=True)
            n2_sb = singles.tile([1, 1], FP32)
            nc.scalar.copy(n2_sb[:], n2_ps[:])
            y0 = singles.tile([1, 1], FP32)
            nc.scalar.sqrt(y0[:], n2_sb[:])
            ry = singles.tile([1, 1], FP32)
            nc.vector.reciprocal(ry[:], y0[:])
            t0 = singles.tile([1, 1], FP32)
            nc.scalar.activation(t0[:], n2_sb[:], ACTF.Copy,
                                 bias=0.0, scale=ry[:])      # n2 / y0
            y2 = singles.tile([1, 1], FP32)
            nc.scalar.activation(y2[:], y0[:], ACTF.Identity,
                                 bias=t0[:], scale=1.0)      # 2*sqrt(n2)
            rn2 = singles.tile([1, 1], FP32)
            nc.vector.reciprocal(rn2[:], y2[:])              # 1/(2 sqrt)
            vs1 = singles.tile([1, 1], FP32)
            nc.scalar.activation(vs1[:], rn2[:], ACTF.Copy,
                                 bias=0.0, scale=scal_a[0:1, 0:1])
            vs = singles.tile([1, 1], FP32)
            nc.scalar.activation(vs[:], vs1[:], ACTF.Copy,
                                 bias=0.0, scale=2.0)        # scal/||a||

            # Sa = sum_h a_h ; SvA = vs * Sa
            a_sum = singles.tile([128, 1], FP32)
            nc.vector.tensor_reduce(a_sum[:], att_v[:], axis=AX.X, op=ALU.add)
            sa_ps = ps_sm.tile([1, 1], FP32, tag="ps")
            nc.tensor.matmul(sa_ps[:], lhsT=a_sum[:], rhs=ones_col[:],
                             start=True, stop=True)
            sa_sb = singles.tile([1, 1], FP32)
            nc.scalar.copy(sa_sb[:], sa_ps[:])
            sva = singles.tile([1, 1], FP32)
            nc.scalar.activation(sva[:], sa_sb[:], ACTF.Copy,
                                 bias=0.0, scale=vs[:])

            # broadcast vs and vs*Sa to 64-partition columns (for exp)
            vs_ps = ps_sm.tile([64, 1], FP32, tag="ps")
            nc.tensor.matmul(vs_ps[:], lhsT=ones_row[0:1, 0:64], rhs=vs[:],
                             start=True, stop=True)
            vs_col = singles.tile([64, 1], FP32)
            nc.scalar.copy(vs_col[:], vs_ps[:])
            sva_ps = ps_sm.tile([64, 1], FP32, tag="ps")
            nc.tensor.matmul(sva_ps[:], lhsT=ones_row[0:1, 0:64], rhs=sva[:],
                             start=True, stop=True)
            sva_col = singles.tile([64, 1], FP32)
            nc.scalar.copy(sva_col[:], sva_ps[:])

            # ---------------- projections ----------------
            # Qp^T[hb] -> [128, 64] (+bias via ACT)
            qpb = singles.tile([128, 2, QC], FP32)
            for hb in range(2):
                q_ps = ps_sm.tile([128, QC], FP32, tag="ps")
                for dch in range(8):
                    nc.tensor.matmul(
                        q_ps[:],
                        lhsT=wqt_sb[:, dch, hb * 128:(hb + 1) * 128],
                        rhs=qt_sb[:, dch, :],
                        start=(dch == 0), stop=(dch == 7))
                nc.scalar.activation(qpb[:, hb, :], q_ps[:], ACTF.Identity,
                                     bias=bias_sb[:, hb:hb + 1], scale=1.0)

            # Ea[hb] = exp(2 * Qp^T)
            ea = singles.tile([128, 2, QC], FP32)
            for hb in range(2):
                nc.scalar.activation(ea[:, hb, :], qpb[:, hb, :], ACTF.Exp,
                                     bias=0.0, scale=2.0)

            # Kp^T[hb] in SBUF, split by k-half so route A (half0) can
            # start before half1's input DMA completes
            kpt_h = [singles.tile([128, 2, 512], FP32, name="kpt_h0"),
                     singles.tile([128, 2, 512], FP32, name="kpt_h1")]
            for half in (1, 0):
                for hb in range(2):
                    kp_ps = ps_sm.tile([128, 512], FP32, tag="ps")
                    for dch in range(8):
                        nc.tensor.matmul(
                            kp_ps[:],
                            lhsT=wkt_sb[:, dch, hb * 128:(hb + 1) * 128],
                            rhs=(kt_h0 if half == 0 else kt_h1)[:, dch, :],
                            start=(dch == 0), stop=(dch == 7))
                    nc.scalar.copy(kpt_h[half][:, hb, :], kp_ps[:])

            # Eb[hb] = exp(2 * Kp^T[:, m:])  (route-B source; m == 512)
            assert m == 512, "tile split assumes m == 512"
            eb = singles.tile([128, 2, w], FP32)
            for hb in range(2):
                nc.scalar.activation(eb[:, hb, :], kpt_h[1][:, hb, :],
                                     ACTF.Exp, bias=0.0, scale=2.0)

            # Kp[k,h] tiles for the context matmul: transpose Kp^T
            kp_kh = singles.tile([128, 8, H], FP32)
            for c in range(8):
                for hb in range(2):
                    tr_ps = ps_sm.tile([128, 128], FP32, tag="ps")
                    src_t = kpt_h[c // 4][:, hb,
                                          (c % 4) * 128:(c % 4 + 1) * 128]
                    nc.tensor.transpose(tr_ps[:], src_t, ident[:])
                    nc.scalar.copy(kp_kh[:, c, hb * 128:(hb + 1) * 128],
                                   tr_ps[:])

            # ---------------- main loop: scores ----------------
            scores = singles.tile([QC, KS], FP32)
            segs_a = _segments(0, m)
            segs_b = _segments(m, KS)
            for g in range(16):
                # separate PSUM tiles per route: a start=True matmul clears
                # accumulation state for its whole (partition-row x bank)
                sc_a = (ps_sc.tile([128, m], FP32, tag="sca", name="sc_a")
                        if m > 0 else None)
                sc_b = (ps_sc.tile([128, w], FP32, tag="scb", name="sc_b")
                        if w > 0 else None)
                tts = {}
                rts = {}
                for hb in range(2):
                    for j in range(4):
                        q = g + 16 * j
                        if m > 0:
                            t_t = pool_a.tile([128, m], BF16, tag="T")
                            nc.scalar.activation(
                                t_t[:], kpt_h[0][:, hb, :], ACTF.Tanh,
                                bias=qpb[:, hb, q:q + 1], scale=1.0)
                            tts[(hb, j)] = t_t
                        if w > 0:
                            r_t = pool_b.tile([128, w], BF16, tag="R")
                            nc.vector._custom_dve(
                                fused_op, out=r_t[:], in0=eb[:, hb, :],
                                s0=ea[:, hb, q:q + 1], s1=MR_SEED, imm2=MR_NR)
                            rts[(hb, j)] = r_t
                # consecutive j-varied matmuls -> 4-way col-group concurrency
                for hb in range(2):
                    for (n0, n1) in segs_a:
                        for j in range(4):
                            nc.tensor.matmul(
                                sc_a[32 * j:32 * j + 1, n0:n1],
                                lhsT=att_r[:, hb:hb + 1],
                                rhs=tts[(hb, j)][:, n0:n1],
                                start=(hb == 0), stop=(hb == 1),
                                tile_position=(0, 32 * j))
                    for (n0, n1) in segs_b:
                        for j in range(4):
                            nc.tensor.matmul(
                                sc_b[32 * j:32 * j + 1, n0 - m:n1 - m],
                                lhsT=attneg2[:, hb:hb + 1],
                                rhs=rts[(hb, j)][:, n0 - m:n1 - m],
                                start=(hb == 0), stop=(hb == 1),
                                tile_position=(0, 32 * j))
                # evacuate PSUM (full-tile engine copies: cost is free-dim
                # based), then DMA-remap the 4 live rows to dense SBUF rows
                scsc = pool_b.tile([128, KS], FP32, tag="scsc")
                if m > 0:
                    nc.scalar.copy(scsc[:, 0:m], sc_a[:])
                if w > 0:
                    nc.vector.tensor_copy(scsc[:, m:KS], sc_b[:])
                for j in range(4):
                    nc.sync.dma_start(
                        out=scores[g + 16 * j:g + 16 * j + 1, :],
                        in_=scsc[32 * j:32 * j + 1, :])

            # ---------------- softmax (no max-subtract: |s| <= 1) --------
            exp_sb = singles.tile([QC, KS], FP32)
            acc_a = singles.tile([QC, 1], FP32)
            acc_b = singles.tile([QC, 1], FP32)
            if m > 0:
                nc.scalar.activation(exp_sb[:, 0:m], scores[:, 0:m], ACTF.Exp,
                                     bias=0.0, scale=vs_col[:],
                                     accum_out=acc_a[:])
            else:
                nc.vector.memset(acc_a[:], 0.0)
            if w > 0:
                # route-B psum was short by Sa; compensate: exp(vs*s + vs*Sa)
                nc.scalar.activation(exp_sb[:, m:KS], scores[:, m:KS],
                                     ACTF.Exp, bias=sva_col[:],
                                     scale=vs_col[:], accum_out=acc_b[:])
            else:
                nc.vector.memset(acc_b[:], 0.0)
            ssum = singles.tile([QC, 1], FP32)
            nc.vector.tensor_add(ssum[:], acc_a[:], acc_b[:])
            rsum = singles.tile([QC, 1], FP32)
            nc.vector.reciprocal(rsum[:], ssum[:])
            probs_sb = singles.tile([QC, KS], FP32)
            nc.vector.tensor_scalar(probs_sb[:], exp_sb[:], rsum[:], None,
                                    ALU.mult)
            nc.sync.dma_start(out=probs_e[:], in_=probs_sb[:])
            if dbg_scores_e is not None:
                nc.sync.dma_start(out=dbg_scores_e[:], in_=scores[:])

            # ---------------- context = probs @ Kp ----------------
            pt_sb = singles.tile([128, 8, QC], FP32)
            for c in range(8):
                pt_ps = ps_sm.tile([128, QC], FP32, tag="ps")
                nc.tensor.transpose(
                    pt_ps[:], probs_sb[:, c * 128:(c + 1) * 128],
                    ident[0:QC, 0:QC])
                nc.scalar.copy(pt_sb[:, c, :], pt_ps[:])
            ctx_ps = ps_sm.tile([QC, H], FP32, tag="ps")
            for c in range(8):
                nc.tensor.matmul(ctx_ps[:],
                                 lhsT=pt_sb[:, c, :],
                                 rhs=kp_kh[:, c, :],
                                 start=(c == 0), stop=(c == 7))
            ctx_sb = singles.tile([QC, H], FP32)
            nc.scalar.copy(ctx_sb[:], ctx_ps[:])
            nc.sync.dma_start(out=ctx_e[:], in_=ctx_sb[:])

    nc.compile()
    return nc


_NC_CACHE = {}


def kernel(Q, K, Wq, Wk, attention, scalar_constant, bias_constant):
    global LAST_RESULTS
    Q = np.asarray(Q, dtype=np.float32)
    K = np.asarray(K, dtype=np.float32)
    Wq = np.asarray(Wq, dtype=np.float32)
    Wk = np.asarray(Wk, dtype=np.float32)
    attention = np.asarray(attention, dtype=np.float32)
    scalar_constant = np.asarray(scalar_constant, dtype=np.float32)
    bias_constant = np.asarray(bias_constant, dtype=np.float32)

    m = M_SPLIT
    if m not in _NC_CACHE:
        _NC_CACHE[m] = build_nc(m)
    nc = _NC_CACHE[m]

    wkt = np.ascontiguousarray(Wk.T)                      # [D, H]
    wqt = np.ascontiguousarray(Wq.T)
    attc = np.ascontiguousarray(attention.reshape(2, 128).T)   # [128, 2]
    biasc = np.ascontiguousarray(bias_constant.reshape(2, 128).T)
    scal = scalar_constant.reshape(1, 1)

    in_maps = []
    for c in range(NCORES):
        b, qoff = c // 2, (c % 2) * QC
        in_maps.append({
            "KT": np.ascontiguousarray(K[b].T),
            "WkT": wkt,
            "WqT": wqt,
            "QT": np.ascontiguousarray(Q[b, qoff:qoff + QC].T),
            "attc": attc,
            "biasc": biasc,
            "scal": scal,
        })

    res = run_bass_kernel_spmd(nc, in_maps, list(range(NCORES)),
                               trace=bool(os.environ.get("ATTN_TRACE")))
    LAST_RESULTS = res

    context = np.empty((B, QS, H), np.float32)
    probs = np.empty((B, QS, KS), np.float32)
    for c in range(NCORES):
        b, qoff = c // 2, (c % 2) * QC
        context[b, qoff:qoff + QC] = res.results[c]["ctx"]
        probs[b, qoff:qoff + QC] = res.results[c]["probs"]
    return context, probs


# revision 20
# speedup vs baseline: 1.0924x; 1.0924x over previous
"""Bahdanau additive attention on 8 trn2 NeuronCores.

Math (per batch b):
  Qp = Q[b] @ Wq.T            [128, 256]
  Kp = K[b] @ Wk.T            [1024, 256]
  v  = attention / ||attention|| * scalar_constant
  s[q,k]   = sum_h v_h * tanh(Qp[q,h] + Kp[k,h] + bias_h)
  probs    = softmax_k(s)
  context  = probs @ Kp

Sharding: 8 cores = (4 batches) x (2 query-halves of 64).

Per-core device strategy (layout: h on partitions, k on free dim):
  - PE: Kp^T/Qp^T projections (Kp in float32r), score reduction over h
    as M=1 bf16 matmuls packed 4-per-group via tile_position col-groups,
    probs^T/Kp transposes, context matmul.
  - The 64*1024*256 elementwise tensor is split at k=M_SPLIT:
      k < M_SPLIT : ACT tanh(Kp^T[:,k] + Qp_col_q) (bias fused), bf16 out
      k >= M_SPLIT: DVE fused custom op: ~1/(Ea*Eb + 1) in ONE pass
        (bitwise-NOT seed + 1 Newton step; Ea=e^{2Qp} as the per-partition
        scalar, Eb=e^{2Kp} precomputed), bf16 out; tanh = 1 - 2R with the
        additive constant sum_h a_h compensated in the softmax exp bias.
  - softmax needs no max-subtraction (|scores| <= 1 provably); the
    normalization scal/||a|| is applied via the ACT-exp `scale` operand,
    so the attention vector itself is used unnormalized as matmul lhsT.
  - PSUM rule: a start=True matmul clears accumulation state for its
    whole (partition-row x bank) => the two routes' accumulation groups
    live in separate PSUM tiles (disjoint banks).
"""

import os
import sys

sys.path.insert(0, "/opt/trn_rl_repo")

import numpy as np

import concourse.bass as bass
import concourse.bacc as bacc
import concourse.tile as tile
from concourse import mybir
from concourse.masks import make_identity
from concourse.bass_utils import run_bass_kernel_spmd

FP32 = mybir.dt.float32
BF16 = mybir.dt.bfloat16
F32R = mybir.dt.float32r
AX = mybir.AxisListType
ALU = mybir.AluOpType
ACTF = mybir.ActivationFunctionType

B, QS, KS = 4, 128, 1024
D, H = 1024, 256
QC = 64           # queries per core
NCORES = 8
M_SPLIT = int(os.environ.get("ATTN_M_SPLIT", "512"))  # k cols on ACT route

# 1-Newton-step approx of 1/(x*s+1), minimax-tuned (max rel err ~1.8e-3,
# below the bf16 output rounding)
MR_SEED = -0.2358
MR_NR = 2.0018334

LAST_RESULTS = None  # test harness peeks at this for exec_time_ns


def _register_fused_recip():
    """Register the custom DVE op out = approx(1/(Src0*s0 + 1)) at runtime."""
    import concourse.dve_ops as dvo
    from concourse.dve_spec import Spec, Src0, C0, C1, C2, One, AluOp, Bin, lower
    from concourse.dve_uop import DveOpSpec

    name = "MADD_RECIP1_ANT"
    for o in dvo.OPS:
        if o.name == name:
            return o
    P = Src0 * C0 + One
    nP = Bin(AluOp.BITWISE_NOT, P, P)
    z0 = nP * C1

    def _ref(in0, in1, c0, c1, c2):
        Pv = (in0 * c0 + 1.0).astype(np.float32)
        nPv = (~Pv.view(np.int32)).view(np.float32)
        z = nPv * np.float32(c1)
        return z * (np.float32(c2) - Pv * z)

    spec = Spec(body=z0 * (C2 - P * z0), reference=_ref)
    row = dvo._CUSTOM_DVE_ROW_BASE + len(dvo.OPS)
    shas = {}
    for ver in ("v3", "v4"):
        s = DveOpSpec(name=name, opcode=row, uops=lower(spec, ver=ver),
                      rd1_en=False)
        shas[ver] = s.sha(ver)
    op = dvo.DveOp(name, spec, subdim=False, uops_sha=shas)
    dvo.OPS.append(op)
    dvo.CUSTOM_DVE_SPECS[name] = spec
    dvo._SUB_OPCODE_FOR_NAME[name] = row
    return op


def _segments(lo, hi):
    """Split [lo,hi) at 512-aligned PSUM bank boundaries."""
    segs = []
    while lo < hi:
        nxt = min(hi, (lo // 512 + 1) * 512)
        segs.append((lo, nxt))
        lo = nxt
    return segs


def build_nc(m=M_SPLIT):
    fused_op = _register_fused_recip()
    nc = bacc.Bacc("TRN2", target_bir_lowering=False, debug=False)
    w = KS - m

    kt_e = nc.dram_tensor("KT", [D, KS], F32R, kind="ExternalInput")
    wkt_e = nc.dram_tensor("WkT", [D, H], F32R, kind="ExternalInput")
    wqt_e = nc.dram_tensor("WqT", [D, H], FP32, kind="ExternalInput")
    qt_e = nc.dram_tensor("QT", [D, QC], FP32, kind="ExternalInput")
    att_e = nc.dram_tensor("attc", [128, 2], FP32, kind="ExternalInput")
    bias_e = nc.dram_tensor("biasc", [128, 2], FP32, kind="ExternalInput")
    scal_e = nc.dram_tensor("scal", [1, 1], FP32, kind="ExternalInput")
    probs_e = nc.dram_tensor("probs", [QC, KS], FP32, kind="ExternalOutput")
    ctx_e = nc.dram_tensor("ctx", [QC, H], FP32, kind="ExternalOutput")
    dbg_scores_e = None
    if os.environ.get("ATTN_DEBUG_SCORES"):
        dbg_scores_e = nc.dram_tensor("dbg_scores", [QC, KS], FP32,
                                      kind="ExternalOutput")

    with tile.TileContext(nc) as tc:
        with (
            tc.tile_pool(name="singles", bufs=1) as singles,
            tc.tile_pool(name="ta", bufs=8) as pool_a,
            tc.tile_pool(name="tb", bufs=8) as pool_b,
            tc.tile_pool(name="ps_sc", bufs=2, space="PSUM") as ps_sc,
            tc.tile_pool(name="ps_sm", bufs=3, space="PSUM") as ps_sm,
        ):
            # ---------------- constants / params ----------------
            ident = singles.tile([128, 128], FP32)
            make_identity(nc, ident[:])
            ones_col = singles.tile([128, 1], FP32)
            nc.vector.memset(ones_col[:], 1.0)
            ones_row = singles.tile([1, 128], FP32)
            nc.vector.memset(ones_row[:], 1.0)

            att_sb = singles.tile([128, 2], FP32)
            nc.sync.dma_start(out=att_sb[:], in_=att_e[:])
            bias_sb = singles.tile([128, 2], FP32)
            nc.sync.dma_start(out=bias_sb[:], in_=bias_e[:])
            scal_sb = singles.tile([1, 1], FP32)
            nc.sync.dma_start(out=scal_sb[:], in_=scal_e[:])

            # small inputs first so the Qp/Ea chain completes early
            qt_sb = singles.tile([128, 8, QC], FP32)
            nc.sync.dma_start(out=qt_sb[:],
                              in_=qt_e.rearrange("(n p) q -> p n q", p=128))
            wqt_sb = singles.tile([128, 8, H], FP32)
            nc.sync.dma_start(out=wqt_sb[:],
                              in_=wqt_e.rearrange("(n p) h -> p n h", p=128))
            wkt_sb = singles.tile([128, 8, H], F32R)
            nc.sync.dma_start(out=wkt_sb[:],
                              in_=wkt_e.rearrange("(n p) h -> p n h", p=128))
            kt_h0 = singles.tile([128, 8, 512], F32R)
            kt_h1 = singles.tile([128, 8, 512], F32R)
            kt_v = kt_e.rearrange("(n p) k -> p n k", p=128)
            for dch in range(8):
                nc.sync.dma_start(out=kt_h1[:, dch, :],
                                  in_=kt_v[:, dch, 512:1024])
            for dch in range(8):
                nc.sync.dma_start(out=kt_h0[:, dch, :],
                                  in_=kt_v[:, dch, 0:512])

            # DVE-domain copy of attention (keeps DVE ops single-wait)
            att_v = singles.tile([128, 2], FP32)
            nc.vector.tensor_copy(att_v[:], att_sb[:])
            # attneg2 = -2 * attention, bf16 (route-B lhsT)
            attneg2 = singles.tile([128, 2], BF16)
            nc.vector.tensor_scalar_mul(attneg2[:], att_v[:], -2.0)
            # bf16 copy of attention (route-A lhsT)
            att_r = singles.tile([128, 2], BF16)
            nc.scalar.copy(att_r[:], att_sb[:])

            # ||a||^2 = sum(a^2) ; vs = scal/sqrt(.) with one Newton step.
            # Scalar chain runs on ACT (scale/bias APs) + DVE reciprocal only,
            # so no DVE op joins two foreign semaphores.
            scal_a = singles.tile([1, 1], FP32)
            nc.scalar.copy(scal_a[:], scal_sb[:])
            asq = singles.tile([128, 2], FP32)
            nc.scalar.square(asq[:], att_sb[:])
            asq_r = singles.tile([128, 1], FP32)
            nc.vector.tensor_reduce(asq_r[:], asq[:], axis=AX.X, op=ALU.add)
            n2_ps = ps_sm.tile([1, 1], FP32, tag="ps")
            nc.tensor.matmul(n2_ps[:], lhsT=asq_r[:], rhs=ones_col[:],
                             start=True, stop=True)
            n2_sb = singles.tile([1, 1], FP32)
            nc.scalar.copy(n2_sb[:], n2_ps[:])
            y0 = singles.tile([1, 1], FP32)
            nc.scalar.sqrt(y0[:], n2_sb[:])
            ry = singles.tile([1, 1], FP32)
            nc.vector.reciprocal(ry[:], y0[:])
            t0 = singles.tile([1, 1], FP32)
            nc.scalar.activation(t0[:], n2_sb[:], ACTF.Copy,
                                 bias=0.0, scale=ry[:])      # n2 / y0
            y2 = singles.tile([1, 1], FP32)
            nc.scalar.activation(y2[:], y0[:], ACTF.Identity,
                                 bias=t0[:], scale=1.0)      # 2*sqrt(n2)
            rn2 = singles.tile([1, 1], FP32)
            nc.vector.reciprocal(rn2[:], y2[:])              # 1/(2 sqrt)
            vs1 = singles.tile([1, 1], FP32)
            nc.scalar.activation(vs1[:], rn2[:], ACTF.Copy,
                                 bias=0.0, scale=scal_a[0:1, 0:1])
            vs = singles.tile([1, 1], FP32)
            nc.scalar.activation(vs[:], vs1[:], ACTF.Copy,
                                 bias=0.0, scale=2.0)        # scal/||a||

            # Sa1 = sum_{h in hb1} a_h ; SvA = vs * Sa1 (the DVE route's
            # dropped additive constant, uniform across rows)
            sa_ps = ps_sm.tile([1, 1], FP32, tag="ps")
            nc.tensor.matmul(sa_ps[:], lhsT=att_v[:, 1:2], rhs=ones_col[:],
                             start=True, stop=True)
            sa_sb = singles.tile([1, 1], FP32)
            nc.scalar.copy(sa_sb[:], sa_ps[:])
            sva = singles.tile([1, 1], FP32)
            nc.scalar.activation(sva[:], sa_sb[:], ACTF.Copy,
                                 bias=0.0, scale=vs[:])

            # broadcast vs and vs*Sa to 64-partition columns (for exp)
            vs_ps = ps_sm.tile([64, 1], FP32, tag="ps")
            nc.tensor.matmul(vs_ps[:], lhsT=ones_row[0:1, 0:64], rhs=vs[:],
                             start=True, stop=True)
            vs_col = singles.tile([64, 1], FP32)
            nc.scalar.copy(vs_col[:], vs_ps[:])
            sva_ps = ps_sm.tile([64, 1], FP32, tag="ps")
            nc.tensor.matmul(sva_ps[:], lhsT=ones_row[0:1, 0:64], rhs=sva[:],
                             start=True, stop=True)
            sva_col = singles.tile([64, 1], FP32)
            nc.scalar.copy(sva_col[:], sva_ps[:])

            # ---------------- projections ----------------
            # Qp^T[hb] -> [128, 64] (+bias via ACT)
            qpb = singles.tile([128, 2, QC], FP32)
            for hb in range(2):
                q_ps = ps_sm.tile([128, QC], FP32, tag="ps")
                for dch in range(8):
                    nc.tensor.matmul(
                        q_ps[:],
                        lhsT=wqt_sb[:, dch, hb * 128:(hb + 1) * 128],
                        rhs=qt_sb[:, dch, :],
                        start=(dch == 0), stop=(dch == 7))
                nc.scalar.activation(qpb[:, hb, :], q_ps[:], ACTF.Identity,
                                     bias=bias_sb[:, hb:hb + 1], scale=1.0)

            # Ea[hb] = exp(2 * Qp^T)
            ea = singles.tile([128, 2, QC], FP32)
            for hb in range(2):
                nc.scalar.activation(ea[:, hb, :], qpb[:, hb, :], ACTF.Exp,
                                     bias=0.0, scale=2.0)

            # Kp^T: one full-k tile per hb. hb1 feeds the DVE route (Eb),
            # hb0 feeds the ACT tanh route.
            kpt_hb = [singles.tile([128, KS], FP32, name="kpt_hb0"),
                      singles.tile([128, KS], FP32, name="kpt_hb1")]
            for hb in (1, 0):
                for half in range(2):
                    kp_ps = ps_sm.tile([128, 512], FP32, tag="ps")
                    for dch in range(8):
                        nc.tensor.matmul(
                            kp_ps[:],
                            lhsT=wkt_sb[:, dch, hb * 128:(hb + 1) * 128],
                            rhs=(kt_h0 if half == 0 else kt_h1)[:, dch, :],
                            start=(dch == 0), stop=(dch == 7))
                    nc.scalar.copy(
                        kpt_hb[hb][:, half * 512:(half + 1) * 512], kp_ps[:])

            # Eb = exp(2 * Kp^T[hb1]) over full k (DVE-route source)
            eb1 = singles.tile([128, KS], FP32)
            nc.scalar.activation(eb1[:], kpt_hb[1][:], ACTF.Exp,
                                 bias=0.0, scale=2.0)

            # Kp[k,h] tiles for the context matmul: transpose Kp^T
            kp_kh = singles.tile([128, 8, H], FP32)
            for c in range(8):
                for hb in range(2):
                    tr_ps = ps_sm.tile([128, 128], FP32, tag="ps")
                    nc.tensor.transpose(
                        tr_ps[:], kpt_hb[hb][:, c * 128:(c + 1) * 128],
                        ident[:])
                    nc.scalar.copy(kp_kh[:, c, hb * 128:(hb + 1) * 128],
                                   tr_ps[:])

            # ---------------- main loop: scores ----------------
            # Per (q, hb) row-block routing: hb0 -> ACT tanh (full k),
            # hb1 -> DVE fused ~1/(Ea*Eb+1) (full k). Each (row, bank)
            # hosts exactly one accumulation group {hb0 start, hb1 stop}.
            scores = singles.tile([QC, KS], FP32)
            for g in range(16):
                sc_ps = ps_sc.tile([128, KS], FP32, tag="sc")
                t0s = {}
                x1s = {}
                for j in range(4):
                    q = g + 16 * j
                    t0 = pool_a.tile([128, KS], BF16, tag="T")
                    nc.scalar.activation(t0[:], kpt_hb[0][:], ACTF.Tanh,
                                         bias=qpb[:, 0, q:q + 1], scale=1.0)
                    t0s[j] = t0
                    x1 = pool_b.tile([128, KS], BF16, tag="R")
                    nc.vector._custom_dve(
                        fused_op, out=x1[:], in0=eb1[:],
                        s0=ea[:, 1, q:q + 1], s1=MR_SEED, imm2=MR_NR)
                    x1s[j] = x1
                for bank in range(2):
                    n0, n1 = bank * 512, (bank + 1) * 512
                    for j in range(4):
                        nc.tensor.matmul(
                            sc_ps[32 * j:32 * j + 1, n0:n1],
                            lhsT=att_r[:, 0:1], rhs=t0s[j][:, n0:n1],
                            start=True, stop=False,
                            tile_position=(0, 32 * j))
                    for j in range(4):
                        nc.tensor.matmul(
                            sc_ps[32 * j:32 * j + 1, n0:n1],
                            lhsT=attneg2[:, 1:2], rhs=x1s[j][:, n0:n1],
                            start=False, stop=True,
                            tile_position=(0, 32 * j))
                # evacuate PSUM, then DMA-remap live rows to dense SBUF rows
                scsc = pool_b.tile([128, KS], FP32, tag="scsc")
                nc.scalar.copy(scsc[:, 0:640], sc_ps[:, 0:640])
                nc.vector.tensor_copy(scsc[:, 640:KS], sc_ps[:, 640:KS])
                for j in range(4):
                    nc.sync.dma_start(
                        out=scores[g + 16 * j:g + 16 * j + 1, :],
                        in_=scsc[32 * j:32 * j + 1, :])

            # ---------------- softmax (no max-subtract: |s| <= 1) --------
            exp_sb = singles.tile([QC, KS], FP32)
            ssum = singles.tile([QC, 1], FP32)
            # psum rows are short by Sa1 (DVE route); compensate in the bias
            nc.scalar.activation(exp_sb[:], scores[:], ACTF.Exp,
                                 bias=sva_col[:], scale=vs_col[:],
                                 accum_out=ssum[:])
            rsum = singles.tile([QC, 1], FP32)
            nc.vector.reciprocal(rsum[:], ssum[:])
            probs_sb = singles.tile([QC, KS], FP32)
            nc.vector.tensor_scalar(probs_sb[:], exp_sb[:], rsum[:], None,
                                    ALU.mult)
            nc.sync.dma_start(out=probs_e[:], in_=probs_sb[:])
            if dbg_scores_e is not None:
                nc.sync.dma_start(out=dbg_scores_e[:], in_=scores[:])

            # ---------------- context = probs @ Kp ----------------
            pt_sb = singles.tile([128, 8, QC], FP32)
            for c in range(8):
                pt_ps = ps_sm.tile([128, QC], FP32, tag="ps")
                nc.tensor.transpose(
                    pt_ps[:], probs_sb[:, c * 128:(c + 1) * 128],
                    ident[0:QC, 0:QC])
                nc.scalar.copy(pt_sb[:, c, :], pt_ps[:])
            ctx_ps = ps_sm.tile([QC, H], FP32, tag="ps")
            for c in range(8):
                nc.tensor.matmul(ctx_ps[:],
                                 lhsT=pt_sb[:, c, :],
                                 rhs=kp_kh[:, c, :],
                                 start=(c == 0), stop=(c == 7))
            ctx_sb = singles.tile([QC, H], FP32)
            nc.scalar.copy(ctx_sb[:], ctx_ps[:])
            nc.sync.dma_start(out=ctx_e[:], in_=ctx_sb[:])

    nc.compile()
    return nc


_NC_CACHE = {}


def kernel(Q, K, Wq, Wk, attention, scalar_constant, bias_constant):
    global LAST_RESULTS
    Q = np.asarray(Q, dtype=np.float32)
    K = np.asarray(K, dtype=np.float32)
    Wq = np.asarray(Wq, dtype=np.float32)
    Wk = np.asarray(Wk, dtype=np.float32)
    attention = np.asarray(attention, dtype=np.float32)
    scalar_constant = np.asarray(scalar_constant, dtype=np.float32)
    bias_constant = np.asarray(bias_constant, dtype=np.float32)

    m = M_SPLIT
    if m not in _NC_CACHE:
        _NC_CACHE[m] = build_nc(m)
    nc = _NC_CACHE[m]

    wkt = np.ascontiguousarray(Wk.T)                      # [D, H]
    wqt = np.ascontiguousarray(Wq.T)
    attc = np.ascontiguousarray(attention.reshape(2, 128).T)   # [128, 2]
    biasc = np.ascontiguousarray(bias_constant.reshape(2, 128).T)
    scal = scalar_constant.reshape(1, 1)

    in_maps = []
    for c in range(NCORES):
        b, qoff = c // 2, (c % 2) * QC
        in_maps.append({
            "KT": np.ascontiguousarray(K[b].T),
            "WkT": wkt,
            "WqT": wqt,
            "QT": np.ascontiguousarray(Q[b, qoff:qoff + QC].T),
            "attc": attc,
            "biasc": biasc,
            "scal": scal,
        })

    res = run_bass_kernel_spmd(nc, in_maps, list(range(NCORES)),
                               trace=bool(os.environ.get("ATTN_TRACE")))
    LAST_RESULTS = res

    context = np.empty((B, QS, H), np.float32)
    probs = np.empty((B, QS, KS), np.float32)
    for c in range(NCORES):
        b, qoff = c // 2, (c % 2) * QC
        context[b, qoff:qoff + QC] = res.results[c]["ctx"]
        probs[b, qoff:qoff + QC] = res.results[c]["probs"]
    return context, probs
